# revision 11
# baseline (speedup 1.0000x reference)
"""Trainium2 Bass kernel for nn_ActorCritic loss_fn.

Strategy (v2, batch-major)
--------------------------
Both losses are polynomials in 10 masked global sums over the discounted
returns R, values V, log-probs L, entropies E and mask m:

    N    = sum(m)        S1   = sum(m*R)      S2  = sum(m*R^2)
    SV   = sum(m*V)      SRV  = sum(m*R*V)    SV2 = sum(m*V^2)
    SLP  = sum(m*L)      SLPR = sum(m*L*R)    SLPV= sum(m*L*V)
    SE   = sum(m*E)

Layout: batch on partitions, time along the free dim, TIME-REVERSED on the
host.  Each core gets 512 batch columns = 4 partition-blocks of 128; each
block's 8192 time steps split into 4 windows of 2048 -> 16 units of
(128, 2048) per core, streamed with a 3-deep ring.

Engines per unit:
  DVE : discounted-return scan as a native tensor_tensor_scan
        (state = gamma*state + r, fp32 state, gamma held as an f32 tile so
        the recurrence matches the reference bit-for-bit in structure),
        chained across windows via initial=prev[:, -1:]; then bf16 2x
        products mR, mV, mL, mRV, mLR (and mLV on some units).
  GPS : mE product every unit; mLV product on the other units.
  PE  : 7 stat reductions (N,S1,SV,SLP,SRV,SLPR,SLPV) as ones-column
        matmuls accumulating into one PSUM bank across all units.
  ACT : Square+accum_out for S2 and SV2, Copy+accum_out for SE
        (one column per unit; host sums).

Raw Bass with manual semaphores (walrus build allows one sync wait per
instruction -> standalone wait_ge).  Final scalar math on host in float64.
"""

import numpy as np
from contextlib import ExitStack

GAMMA = 0.99
ALPHA = 0.01
EPS = 1e-8

T = 8192
B = 4096
NCORES = 8
BL = B // NCORES          # 512 batch columns per core
P = 128                   # partition dim (batch block)
NBLK = BL // P            # 4 batch blocks
W = 2048                  # time window (free dim per unit)
NWIN = T // W             # 4 windows per block
NUNIT = NBLK * NWIN       # 16 units, u = j*NWIN + w
NCH = W // 512            # 4 matmul chunks per unit (moving max 512)

# dtypes for rewards / entropies ("bf16" or "fp8")
R_DT = "fp8"
E_DT = "bf16"
# GPS tensor_tensor ops grab the SBUF port pair that DVE's 2x perf mode
# needs (exclusive lock per instruction), so GPS gets ONLY the mE product,
# gated to run inside DVE's scan window (the scan is a 1x op on DVE's
# dedicated port).  Everything else stays on DVE at 2x.
LV_ON_GPS = [False for u in range(NUNIT)]

PE_STATS = ("N", "S1", "SV", "SLP", "SRV", "SLPR", "SLPV")
NPE = len(PE_STATS)

_cache = {}


def _build_program():
    import concourse.bass as bass
    import concourse.mybir as mybir
    import ml_dtypes

    dt = mybir.dt
    f32 = dt.float32
    bf16 = dt.bfloat16
    fp8 = dt.float8e4
    mult = mybir.AluOpType.mult
    add = mybir.AluOpType.add
    Square = mybir.ActivationFunctionType.Square
    Copy = mybir.ActivationFunctionType.Copy

    r_dt = fp8 if R_DT == "fp8" else bf16
    e_dt = fp8 if E_DT == "fp8" else bf16

    nc = bass.Bass()
    r_d = nc.dram_tensor("rewards", [NUNIT * P, W], r_dt, kind="ExternalInput")
    v_d = nc.dram_tensor("value_estimates", [NUNIT * P, W], bf16, kind="ExternalInput")
    l_d = nc.dram_tensor("log_probs", [NUNIT * P, W], bf16, kind="ExternalInput")
    e_d = nc.dram_tensor("entropies", [NUNIT * P, W], e_dt, kind="ExternalInput")
    m_d = nc.dram_tensor("to_include", [NUNIT * P, W], bf16, kind="ExternalInput")
    pes_d = nc.dram_tensor("pe_stats", [NPE, BL], f32, kind="ExternalOutput")
    cols_d = nc.dram_tensor("acc_cols", [P, 3 * NUNIT], f32, kind="ExternalOutput")

    # onehot matrix for stat matmuls: oneh[:, j*NPE + j] = 1
    oneh_np = np.zeros((P, NPE * NPE), dtype=np.float32)
    for j in range(NPE):
        oneh_np[:, j * NPE + j] = 1.0
    oneh_d = nc.inline_tensor(oneh_np.astype(ml_dtypes.bfloat16), "onehmat")
    # gamma tile for the scan (f32 so the recurrence coefficient is exact)
    gam_d = nc.inline_tensor(np.full((P, W), GAMMA, dtype=np.float32), "gammat")

    with ExitStack() as ctx:
        def sb(name, shape, dtype):
            return ctx.enter_context(nc.sbuf_tensor(name, list(shape), dtype))

        oneh_sb = sb("oneh_sb", (P, NPE * NPE), bf16)
        gam_sb = sb("gam_sb", (P, W), f32)
        r_in = [sb(f"r_in{i}", (P, W), r_dt) for i in range(3)]
        v_in = [sb(f"v_in{i}", (P, W), bf16) for i in range(3)]
        l_in = [sb(f"l_in{i}", (P, W), bf16) for i in range(3)]
        e_in = [sb(f"e_in{i}", (P, W), e_dt) for i in range(3)]
        m_in = [sb(f"m_in{i}", (P, W), bf16) for i in range(3)]
        R_t = [sb(f"R_t{i}", (P, W), bf16) for i in range(2)]
        mR = [sb(f"mR{i}", (P, W), bf16) for i in range(2)]
        mV = [sb(f"mV{i}", (P, W), bf16) for i in range(2)]
        mL = [sb(f"mL{i}", (P, W), bf16) for i in range(2)]
        mRV = [sb(f"mRV{i}", (P, W), bf16) for i in range(2)]
        mLR = [sb(f"mLR{i}", (P, W), bf16) for i in range(2)]
        mLV = [sb(f"mLV{i}", (P, W), bf16) for i in range(2)]
        mE = [sb(f"mE{i}", (P, W), bf16) for i in range(2)]
        sq = sb("sq", (P, W), bf16)
        cols = sb("cols", (P, 3 * NUNIT), f32)
        stats_sb = sb("stats_sb", (NPE, BL), f32)
        st_ps = ctx.enter_context(nc.psum_tensor("st_ps", [NPE, BL], f32))

        with nc.Block() as block, \
                nc.semaphore("const_sem") as const_sem, \
                nc.semaphore("rsem0") as rsem0, \
                nc.semaphore("dr0") as dr0, \
                nc.semaphore("dr1") as dr1, \
                nc.semaphore("dr2") as dr2, \
                nc.semaphore("dve_p8") as dve_p8, \
                nc.semaphore("pe_stat") as pe_stat, \
                nc.semaphore("act_done") as act_done, \
                nc.semaphore("act_se") as act_se, \
                nc.semaphore("act_fin") as act_fin, \
                nc.semaphore("dma_out") as dma_out:
            dring = (dr0, dr1, dr2)
            # per-slot completion thresholds (unit 0's rewards use rsem0)
            thresh = {}
            cnt = [0, 0, 0]
            for u in range(NUNIT):
                cnt[u % 3] += 64 if u == 0 else 80
                thresh[u] = cnt[u % 3]

            @block.sync
            def _(sync):
                # consts + unit-0 rewards first and ALONE, so the first scan
                # is not queued behind the bulk prefetch (queues share fairly)
                sync.dma_start(out=gam_sb[:], in_=gam_d[:]).then_inc(const_sem, 16)
                sync.dma_start(out=oneh_sb[:], in_=oneh_d[:]).then_inc(const_sem, 16)
                sync.dma_start(out=r_in[0][:], in_=r_d[0:P, :]).then_inc(rsem0, 16)
                for dst, src in ((v_in[0], v_d), (l_in[0], l_d),
                                 (e_in[0], e_d), (m_in[0], m_d)):
                    sync.dma_start(out=dst[:], in_=src[0:P, :]).then_inc(dr0, 16)
                sync.wait_ge(dr0, 64)
                for u in range(1, NUNIT):
                    if u >= 3:
                        sync.wait_ge(dve_p8, 8 * (u - 2))
                        sync.wait_ge(pe_stat, u - 2)
                    sl = u % 3
                    rows = slice(u * P, (u + 1) * P)
                    for dst, src in ((r_in[sl], r_d), (v_in[sl], v_d),
                                     (l_in[sl], l_d), (e_in[sl], e_d),
                                     (m_in[sl], m_d)):
                        sync.dma_start(out=dst[:], in_=src[rows, :]) \
                            .then_inc(dring[sl], 16)
                sync.wait_ge(act_fin, 1)
                sync.dma_start(out=pes_d[:], in_=stats_sb[:]).then_inc(dma_out, 16)
                sync.wait_ge(act_done, 2 * NUNIT)
                sync.wait_ge(act_se, NUNIT)
                sync.dma_start(out=cols_d[:], in_=cols[:]).then_inc(dma_out, 16)
                sync.wait_ge(dma_out, 32)

            @block.vector
            def _(dve):
                dve.wait_ge(const_sem, 32)   # both const DMAs (order across queues not guaranteed)
                for u in range(NUNIT):
                    sl = u % 3
                    pr = u % 2
                    if u == 0:
                        dve.wait_ge(rsem0, 16)
                    else:
                        dve.wait_ge(dring[sl], thresh[u])
                    if u >= 2:
                        # product ring WAR: PE stats / ACT reads of u-2 done
                        dve.wait_ge(pe_stat, u - 1)
                        dve.wait_ge(act_done, 2 * (u - 1))
                        dve.wait_ge(act_se, u - 1)
                    init = 0.0 if u % NWIN == 0 else R_t[(u - 1) % 2][:, W - 1:W]
                    dve.tensor_tensor_scan(out=R_t[pr][:], data0=gam_sb[:],
                                           data1=r_in[sl][:], initial=init,
                                           op0=mult, op1=add).then_inc(dve_p8, 1)
                    if u == 0:
                        dve.wait_ge(dring[0], 64)
                    dve.tensor_tensor(out=mR[pr][:], in0=m_in[sl][:], in1=R_t[pr][:], op=mult).then_inc(dve_p8, 1)
                    dve.tensor_tensor(out=mV[pr][:], in0=m_in[sl][:], in1=v_in[sl][:], op=mult).then_inc(dve_p8, 1)
                    dve.tensor_tensor(out=mL[pr][:], in0=m_in[sl][:], in1=l_in[sl][:], op=mult).then_inc(dve_p8, 1)
                    dve.tensor_tensor(out=mRV[pr][:], in0=mR[pr][:], in1=mV[pr][:], op=mult).then_inc(dve_p8, 1)
                    dve.tensor_tensor(out=mLR[pr][:], in0=mL[pr][:], in1=mR[pr][:], op=mult).then_inc(dve_p8, 1)
                    dve.tensor_tensor(out=mLV[pr][:], in0=mL[pr][:], in1=mV[pr][:], op=mult).then_inc(dve_p8, 1)
                    dve.tensor_tensor(out=mE[pr][:], in0=m_in[sl][:], in1=e_in[sl][:], op=mult).then_inc(dve_p8, 1)

            @block.tensor
            def _(pe):
                pe.wait_ge(const_sem, 32)
                # stat j ready after dve_p8 >= 8u+1+prod_idx[j] (N needs only DMA)
                need = {"N": None, "S1": 2, "SV": 3, "SLP": 4,
                        "SRV": 5, "SLPR": 6, "SLPV": 7}
                for u in range(NUNIT):
                    sl = u % 3
                    pr = u % 2
                    srcs = {"N": m_in[sl], "S1": mR[pr], "SV": mV[pr],
                            "SLP": mL[pr], "SRV": mRV[pr], "SLPR": mLR[pr],
                            "SLPV": mLV[pr]}
                    for j, stat in enumerate(PE_STATS):
                        if need[stat] is None:
                            if u == 0:
                                pe.wait_ge(dring[0], 64)
                            else:
                                pe.wait_ge(dring[sl], thresh[u])
                        else:
                            pe.wait_ge(dve_p8, 8 * u + need[stat])
                        for k in range(NCH):
                            csl = slice(k * 512, (k + 1) * 512)
                            mm = pe.matmul(
                                st_ps[:],
                                lhsT=oneh_sb[:, j * NPE:(j + 1) * NPE],
                                rhs=srcs[stat][:, csl],
                                start=(u == 0 and j == 0 and k == 0),
                                stop=(u == NUNIT - 1 and j == NPE - 1 and k == NCH - 1))
                    mm.then_inc(pe_stat, 1)

            @block.scalar
            def _(act):
                for u in range(NUNIT):
                    pr = u % 2
                    act.wait_ge(dve_p8, 8 * u + 2)
                    act.activation(sq[:], mR[pr][:], Square,
                                   accum_out=cols[:, u:u + 1]).then_inc(act_done, 1)
                    act.wait_ge(dve_p8, 8 * u + 3)
                    act.activation(sq[:], mV[pr][:], Square,
                                   accum_out=cols[:, NUNIT + u:NUNIT + u + 1]) \
                        .then_inc(act_done, 1)
                    act.wait_ge(dve_p8, 8 * u + 8)
                    act.activation(sq[:], mE[pr][:], Copy,
                                   accum_out=cols[:, 2 * NUNIT + u:2 * NUNIT + u + 1]) \
                        .then_inc(act_se, 1)
                act.wait_ge(pe_stat, NUNIT)
                act.activation(stats_sb[:], st_ps[:], Copy).then_inc(act_fin, 1)

    return nc


def _get_program():
    if "nc" not in _cache:
        _cache["nc"] = _build_program()
    return _cache["nc"]


def _tile(x, dtype):
    """(T, BL) shard -> time-reversed, batch-major unit layout (NUNIT*P, W)."""
    # reverse time, transpose to (BL, T)
    xt = x[::-1, :].T                              # (BL, T)
    xt = xt.reshape(NBLK, P, NWIN, W)              # (j, p, w, t)
    xt = np.ascontiguousarray(xt.transpose(0, 2, 1, 3), dtype=np.float32)
    return xt.reshape(NUNIT * P, W).astype(dtype)


def _shard_inputs(inputs):
    import ml_dtypes

    bf16 = ml_dtypes.bfloat16
    fp8 = ml_dtypes.float8_e4m3
    r_t = fp8 if R_DT == "fp8" else bf16
    e_t = fp8 if E_DT == "fp8" else bf16

    r = np.asarray(inputs["rewards"], dtype=np.float32)
    v = np.asarray(inputs["value_estimates"], dtype=np.float32)
    lp = np.asarray(inputs["log_probs"], dtype=np.float32)
    e = np.asarray(inputs["entropies"], dtype=np.float32)
    m = inputs["to_include"].astype(np.float32)
    in_maps = []
    for c in range(NCORES):
        sl = slice(c * BL, (c + 1) * BL)
        in_maps.append({
            "rewards": _tile(r[:, sl], r_t),
            "value_estimates": _tile(v[:, sl], bf16),
            "log_probs": _tile(lp[:, sl], bf16),
            "entropies": _tile(e[:, sl], e_t),
            "to_include": _tile(m[:, sl], bf16),
        })
    return in_maps


def _execute(in_maps, trace=False):
    from concourse.bass_utils import run_bass_kernel_spmd

    nc = _get_program()
    return run_bass_kernel_spmd(nc, in_maps, list(range(NCORES)), trace=trace)


def _stats_from_results(results):
    tot = {name: 0.0 for name in PE_STATS + ("S2", "SV2", "SE")}
    for cm in results:
        pes = cm["pe_stats"].astype(np.float64)
        for j, name in enumerate(PE_STATS):
            tot[name] += pes[j].sum()
        ac = cm["acc_cols"].astype(np.float64)
        tot["S2"] += ac[:, 0:NUNIT].sum()
        tot["SV2"] += ac[:, NUNIT:2 * NUNIT].sum()
        tot["SE"] += ac[:, 2 * NUNIT:3 * NUNIT].sum()
    return tot


def _finalize(tot):
    N = tot["N"]; S1 = tot["S1"]; S2 = tot["S2"]
    SV = tot["SV"]; SRV = tot["SRV"]; SV2 = tot["SV2"]
    SLP = tot["SLP"]; SLPR = tot["SLPR"]; SLPV = tot["SLPV"]; SE = tot["SE"]
    mean = S1 / N
    q = S2 - 2.0 * mean * S1 + mean * mean * N   # sum(m*(R-mean)^2)
    var = q / (N - 1.0)
    s = np.sqrt(var) + EPS
    critic = q / (s * s) - 2.0 * (SRV - mean * SV) / s + SV2
    actor = -(SLPR - mean * SLP) / s + SLPV - ALPHA * SE
    return (np.float32(critic), np.float32(actor))


def kernel(**inputs):
    in_maps = _shard_inputs(inputs)
    res = _execute(in_maps, trace=False)
    tot = _stats_from_results(res.results)
    return _finalize(tot)
